# revision 1
# baseline (speedup 1.0000x reference)
"""Trainium2 Bass kernel for nn_BMSampling: out = X.reshape(B*C, T) @ smp_weight.

Strategy:
- smp_weight columns are interpolation stencils; ~55.6% are entirely zero,
  so their output columns are exactly 0.0. The kernel compacts to nonzero
  columns at runtime (generic for any weight), computes only those on
  device, and scatters into a zero-filled full output on the host.
- Tensor-parallel over the compacted output columns: 8 cores x NSH each.
  Each core computes OUT_shard[512, NSH] = X^T[100,512].T @ W_shard[100,NSH].
- The kernel is output-DMA bound. To keep the PE off the critical path, the
  fp32 matmul (1/4 bf16 rate on trn2: 2 passes x 2 cycles/col) is replaced
  by a 3-term split-fp16 matmul: X = Xh+Xl, W = Wh+Wl (hi/lo fp16 pairs
  built on host), OUT = Xh@Wh + Xl@Wh + Xh@Wl accumulated in fp32 PSUM.
  Dropped Xl@Wl term is ~2^-22; end-to-end error ~1e-7 of scale. DMA bytes
  unchanged (two fp16 halves = one fp32).
- W loads ride the ACT HWDGE ring, output stores the SP ring (no
  head-of-line blocking); the first chunk streams in as 500-col strips so
  the PE starts ~2us in.
"""

from contextlib import ExitStack

import numpy as np

import concourse.bacc as bacc
import concourse.mybir as mybir
import concourse.tile as tile
from concourse import bass_utils

B, C, T = 4, 128, 100
N_SMP, D_PROP = 32, 100
M = B * C                     # 512 matmul rows
NDT = N_SMP * D_PROP * T      # 320000 output columns
NCORES = 8
GRANULE = 1000 * NCORES       # compact col count padded to this

K = T                         # 100 contraction dim (on SBUF partitions)
N_OUTER = 4000                # columns per W tile / output staging tile
N_INNER = 500                 # matmul free dim (fits one PSUM bank: <=512 f32)
F32 = mybir.dt.float32
F16 = mybir.dt.float16

_PROGRAMS = {}


def _build(nsh):
    """Per-core program computing OUT[512, nsh] = XT.T @ W[100, nsh]."""
    if nsh in _PROGRAMS:
        return _PROGRAMS[nsh]

    widths = [N_OUTER] * (nsh // N_OUTER)
    if nsh % N_OUTER:
        widths.append(nsh % N_OUTER)
    assert all(w % (2 * N_INNER) == 0 for w in widths), widths

    nc = bacc.Bacc("TRN2", debug=False)
    xhl = nc.dram_tensor("XHL", [2, K, M], F16, kind="ExternalInput").ap()
    whl = nc.dram_tensor("WHL", [2, K, nsh], F16, kind="ExternalInput").ap()
    out = nc.dram_tensor("OUT", [M, nsh], F32, kind="ExternalOutput").ap()

    with tile.TileContext(nc) as tc, ExitStack() as ctx:
        xpool = ctx.enter_context(tc.tile_pool(name="x", bufs=1))
        wpool = ctx.enter_context(tc.tile_pool(name="w", bufs=4))
        w0pool = ctx.enter_context(tc.tile_pool(name="w0", bufs=8))
        opool = ctx.enter_context(tc.tile_pool(name="o", bufs=4))
        pspool = ctx.enter_context(tc.tile_pool(name="ps", bufs=4, space="PSUM"))

        x_sb = xpool.tile([K, 2, M], F16)
        nc.scalar.dma_start(out=x_sb[:], in_=xhl.rearrange("a k m -> k a m"))
        xh_sb = x_sb[:, 0]
        xl_sb = x_sb[:, 1]

        n0 = 0
        for it, width in enumerate(widths):
            nj = width // N_INNER
            if it == 0:
                # First chunk streams in as 500-col strips so the first
                # matmul starts ~2us in instead of waiting on a 1.6 MB load.
                w_strips = []
                for j in range(nj):
                    sl = slice(n0 + j * N_INNER, n0 + (j + 1) * N_INNER)
                    ws = w0pool.tile([K, 2, N_INNER], F16, tag="w0")
                    nc.scalar.dma_start(
                        out=ws[:], in_=whl[:, :, sl].rearrange("a k n -> k a n")
                    )
                    w_strips.append(ws)
                strip = lambda j: (w_strips[j][:, 0], w_strips[j][:, 1])
            else:
                w_sb = wpool.tile([K, 2, N_OUTER], F16, tag="w_sb")
                nc.scalar.dma_start(
                    out=w_sb[:, :, :width],
                    in_=whl[:, :, n0 : n0 + width].rearrange("a k n -> k a n"),
                )
                strip = lambda j, a=w_sb: (
                    a[:, 0, j * N_INNER : (j + 1) * N_INNER],
                    a[:, 1, j * N_INNER : (j + 1) * N_INNER],
                )
            for m in range(M // 128):
                msl = slice(m * 128, (m + 1) * 128)
                o_sb = opool.tile([128, N_OUTER], F32, tag="o_sb")
                for j in range(0, nj, 2):
                    ps = pspool.tile([128, 2, 512], F32)  # one PSUM bank per slot
                    for h in range(2):
                        wsh, wsl = strip(j + h)
                        dst = ps[:, h, :N_INNER]
                        nc.tensor.matmul(
                            dst, xh_sb[:, msl], wsh, start=True, stop=False
                        )
                        nc.tensor.matmul(
                            dst, xl_sb[:, msl], wsh, start=False, stop=False
                        )
                        nc.tensor.matmul(
                            dst, xh_sb[:, msl], wsl, start=False, stop=True
                        )
                    nc.vector.tensor_copy(
                        out=o_sb[
                            :, j * N_INNER : (j + 2) * N_INNER
                        ].rearrange("p (a b) -> p a b", a=2),
                        in_=ps[:, :, :N_INNER],
                    )
                nc.sync.dma_start(
                    out=out[msl, n0 : n0 + width],
                    in_=o_sb[:, :width],
                )
            n0 += width

    nc.compile()
    _PROGRAMS[nsh] = nc
    return nc


def _split16(a):
    hi = a.astype(np.float16)
    lo = (a - hi.astype(np.float32)).astype(np.float16)
    return np.ascontiguousarray(hi), np.ascontiguousarray(lo)


def prepare_run(X, smp_weight):
    """Returns (nc, in_maps, assemble) where assemble(results)->full output."""
    X = np.ascontiguousarray(np.asarray(X, dtype=np.float32))
    Wfull = np.asarray(smp_weight, dtype=np.float32)

    # Compact away all-zero weight columns: their outputs are exactly 0.0.
    nz = np.flatnonzero((Wfull != 0).any(axis=0))
    padded = max(GRANULE, (len(nz) + GRANULE - 1) // GRANULE * GRANULE)
    nsh = padded // NCORES
    Wc = np.zeros((K, padded), dtype=np.float32)
    Wc[:, : len(nz)] = Wfull[:, nz]

    xt = np.ascontiguousarray(X.reshape(M, T).T)  # [100, 512]
    xhl = np.ascontiguousarray(np.stack(_split16(xt)))        # [2, 100, 512]
    whl = np.stack(_split16(Wc))                              # [2, 100, padded]
    in_maps = [
        {
            "XHL": xhl,
            "WHL": np.ascontiguousarray(whl[:, :, i * nsh : (i + 1) * nsh]),
        }
        for i in range(NCORES)
    ]
    nc = _build(nsh)

    def assemble(results):
        compact = np.concatenate([results[i]["OUT"] for i in range(NCORES)], axis=1)
        full = np.zeros((M, NDT), dtype=np.float32)
        full[:, nz] = compact[:, : len(nz)]
        return full.reshape(B, C, N_SMP, D_PROP, T)

    return nc, in_maps, assemble


def kernel(X, smp_weight):
    nc, in_maps, assemble = prepare_run(X, smp_weight)
    res = bass_utils.run_bass_kernel_spmd(nc, in_maps, core_ids=list(range(NCORES)))
    return assemble(res.results)



# revision 3
# speedup vs baseline: 6.9486x; 6.9486x over previous
"""Trainium2 Bass kernel for nn_BMSampling: out = X.reshape(B*C, T) @ smp_weight.

Strategy:
- smp_weight columns are interpolation stencils. ~55.6% are entirely zero
  (their outputs are exactly 0.0), and the nonzero columns repeat heavily:
  the sample positions are rationals with a small denominator, so only a few
  thousand DISTINCT columns exist (~6k of 320k for the shipped weight).
  Identical weight columns produce bitwise-identical output columns, so the
  kernel dedups columns at runtime (generic for any weight), computes one
  representative of each distinct column on device, and expands via an exact
  gather on the host (same move as scattering the all-zero columns).
- Tensor-parallel over the distinct columns: 8 cores x nsh each. Each core
  computes OUT[512, nsh] = XT[100,512].T @ W[100, nsh].
- fp16 inputs / fp16 output (PSUM accumulates fp32; the PSUM->SBUF copy
  downcasts). Worst-case end-to-end error ~5e-4 of scale, well inside the
  2e-2 gate, and it halves every DMA byte moved.
"""

from contextlib import ExitStack

import numpy as np

import concourse.bacc as bacc
import concourse.mybir as mybir
import concourse.tile as tile
from concourse import bass_utils

B, C, T = 4, 128, 100
N_SMP, D_PROP = 32, 100
M = B * C                     # 512 matmul rows
NDT = N_SMP * D_PROP * T      # 320000 output columns
NCORES = 8
K = T                         # 100 contraction dim (SBUF partitions)

N_CHUNK = 4096                # cols per W tile / output staging tile
N_INNER = 512                 # matmul free dim (one PSUM bank: <=512 f32)
F32 = mybir.dt.float32
F16 = mybir.dt.float16

_PROGRAMS = {}


def _strips(width):
    out, j = [], 0
    while j < width:
        out.append((j, min(N_INNER, width - j)))
        j += N_INNER
    return out


def _build(nsh):
    """Per-core program computing OUT[512, nsh] = XT.T @ W[100, nsh], f16."""
    if nsh in _PROGRAMS:
        return _PROGRAMS[nsh]

    nc = bacc.Bacc("TRN2", debug=False)
    xt = nc.dram_tensor("XT", [K, M], F16, kind="ExternalInput").ap()
    w = nc.dram_tensor("W", [K, nsh], F16, kind="ExternalInput").ap()
    out = nc.dram_tensor("OUT", [M, nsh], F16, kind="ExternalOutput").ap()

    with tile.TileContext(nc) as tc, ExitStack() as ctx:
        xpool = ctx.enter_context(tc.tile_pool(name="x", bufs=1))
        wpool = ctx.enter_context(tc.tile_pool(name="w", bufs=2))
        opool = ctx.enter_context(tc.tile_pool(name="o", bufs=4))
        pspool = ctx.enter_context(tc.tile_pool(name="ps", bufs=8, space="PSUM"))

        x_sb = xpool.tile([K, M], F16)
        nc.scalar.dma_start(out=x_sb[:], in_=xt)

        for n0 in range(0, nsh, N_CHUNK):
            width = min(N_CHUNK, nsh - n0)
            w_sb = wpool.tile([K, N_CHUNK], F16, tag="w_sb")
            nc.scalar.dma_start(out=w_sb[:, :width], in_=w[:, n0 : n0 + width])
            for m in range(M // 128):
                msl = slice(m * 128, (m + 1) * 128)
                o_sb = opool.tile([128, N_CHUNK], F16, tag="o_sb")
                for j0, wdt in _strips(width):
                    ps = pspool.tile([128, N_INNER], F32)
                    nc.tensor.matmul(
                        ps[:, :wdt],
                        x_sb[:, msl],
                        w_sb[:, j0 : j0 + wdt],
                        start=True,
                        stop=True,
                    )
                    # split the PSUM->SBUF downcast copies across DVE and ACT
                    if (j0 // N_INNER) % 2 == 0:
                        nc.vector.tensor_copy(
                            out=o_sb[:, j0 : j0 + wdt], in_=ps[:, :wdt]
                        )
                    else:
                        nc.scalar.copy(out=o_sb[:, j0 : j0 + wdt], in_=ps[:, :wdt])
                nc.sync.dma_start(
                    out=out[msl, n0 : n0 + width], in_=o_sb[:, :width]
                )

    nc.compile()
    _PROGRAMS[nsh] = nc
    return nc


def _dedup_cols(Wnz):
    """Return (first_idx, inv) deduplicating the columns of Wnz [K, n]."""
    n = Wnz.shape[1]
    # Fast path: every column is a <=2-tap adjacent-row stencil, so the
    # triple (first_row, v0, v1) is a complete key. Verified exactly below.
    r0 = np.argmax(Wnz != 0, axis=0)
    ar = np.arange(n)
    v0 = Wnz[r0, ar]
    has2 = r0 + 1 < K
    v1 = np.where(has2, Wnz[np.minimum(r0 + 1, K - 1), ar], 0.0)
    Wrec = np.zeros_like(Wnz)
    Wrec[r0, ar] = v0
    Wrec[r0[has2] + 1, ar[has2]] += v1[has2]
    if np.array_equal(Wrec, Wnz):
        keys = np.empty((n, 3), np.float32)
        keys[:, 0] = r0
        keys[:, 1] = v0
        keys[:, 2] = v1
        kv = np.ascontiguousarray(keys).view("V12").ravel()
    else:  # generic (any structure): key on full column bytes
        kv = np.ascontiguousarray(Wnz.T).view(f"V{4 * Wnz.shape[0]}").ravel()
    _, first_idx, inv = np.unique(kv, return_index=True, return_inverse=True)
    return first_idx, inv


def prepare_run(X, smp_weight):
    """Returns (nc, in_maps, assemble) where assemble(results)->full output."""
    X = np.asarray(X, dtype=np.float32)
    Wfull = np.asarray(smp_weight, dtype=np.float32)

    nz = np.flatnonzero((Wfull != 0).any(axis=0))
    Wnz = Wfull[:, nz]
    first_idx, inv = _dedup_cols(Wnz)
    nu = len(first_idx)

    grain = NCORES * 128
    padded = max(grain, (nu + grain - 1) // grain * grain)
    nsh = padded // NCORES
    Wc = np.zeros((K, padded), dtype=np.float16)
    Wc[:, :nu] = Wnz[:, first_idx]

    xt16 = np.ascontiguousarray(X.reshape(M, T).T.astype(np.float16))
    in_maps = [
        {
            "XT": xt16,
            "W": np.ascontiguousarray(Wc[:, i * nsh : (i + 1) * nsh]),
        }
        for i in range(NCORES)
    ]
    nc = _build(nsh)

    def assemble(results):
        compact = np.concatenate(
            [results[i]["OUT"] for i in range(NCORES)], axis=1
        )
        ext = np.zeros((M, nu + 1), np.float32)
        ext[:, :nu] = compact[:, :nu]
        full_map = np.full(NDT, nu, np.intp)
        full_map[nz] = inv
        full = np.take(ext, full_map, axis=1)
        return full.reshape(B, C, N_SMP, D_PROP, T)

    return nc, in_maps, assemble


def kernel(X, smp_weight):
    nc, in_maps, assemble = prepare_run(X, smp_weight)
    res = bass_utils.run_bass_kernel_spmd(nc, in_maps, core_ids=list(range(NCORES)))
    return assemble(res.results)
